# revision 19
# baseline (speedup 1.0000x reference)
"""Trainium2 Bass kernel for GCN-biased sparse attention (nn_Attention_37589553775245).

Reference computation (per batch b of 8, one NeuronCore each):
    qkv = x @ w_qkv; q,k,v per head (H=8, DH=64)
    attn = softmax(q k^T / sqrt(DH)) + A_hat        (A_hat = D^-1/2 (ceil(adj)+I) D^-1/2)
    out = (attn @ v) @ w_out + b_out

Sharding: pure batch-parallel across the 8 cores (B=8). A_hat is computed on
host (cheap) and replicated; weights replicated. No collectives.

Device-side layout strategy (all matmuls in float32r — tf32-class precision,
1 cycle/row at free-dim 512):
  - x is pre-transposed on host to xT [DIM, N] so the first matmul contraction
    (over DIM) sits on the partition axis.
  - q,k are produced transposed (qT,kT [DH, N]); v in natural [N, F] layout.
  - scores are computed transposed: sT[j,i] = sum_d k[j,d] qT[d,i], so the
    softmax denominator (sum over j) rides the attn@v matmul via an augmented
    V with a ones column: [v_h | 1] gives out rows 0..63 = (exp sT)^T v and
    row 64 = sum_j exp sT[j,i] (the denominator). Softmax max-subtraction is
    skipped: logits*scale ~ N(0,1), exp is safe in fp32.
  - adjacent heads (2h, 2h+1) sit at partition bases 0/64 of one qkT tile, so
    their score matmuls land on disjoint PE row-groups and run concurrently.
  - post-softmax bias contribution A_hat @ V is its own matmul (shared over
    heads), computed transposed via lhsT=V tiles, rhs=A_hat^T (host-shipped),
    interleaved with the attention units that add into the same yT tiles.
  - Y^T = (expv^T * recip_denom_bcast) + (A_hat V)^T accumulates in SBUF, then
    out = Y @ w_out + b_out with lhsT = Y^T tiles.
"""

import os
import sys

import numpy as np

for _p in ("/opt/trn_rl_repo", "/root/.axon_site/_ro/trn_rl_repo"):
    if _p not in sys.path and os.path.isdir(_p):
        sys.path.insert(0, _p)

import concourse.bass as bass  # noqa: E402
import concourse.mybir as mybir  # noqa: E402
import concourse.tile as tile  # noqa: E402
from concourse import bacc  # noqa: E402
from concourse.bass_utils import run_bass_kernel_spmd  # noqa: E402

B, N, DIM, H, DH = 8, 1024, 512, 8, 64
F = H * DH          # 512, inner dim
NT = N // 128       # 8 n-tiles (also j-tiles)
DT = DIM // 128     # 4 dim-tiles
FT = F // 128       # 4 f-tiles
NC2 = N // 512      # 2 i-chunks of 512
SCALE = DH ** -0.5

F32 = mybir.dt.float32
F32R = mybir.dt.float32r

_PROGRAM = None
_last_in_maps = None


def _build_program(reps=1, copies_dve=True, exp_batch=2, interleave=True,
                   pair_heads=False, skip_exp=False, skip_attn=False,
                   mm_bufs=2, s_bufs=None, o_bufs=None, exps_bufs=6):
    nc = bacc.Bacc("TRN2", target_bir_lowering=False, debug=False, num_devices=8)

    xT_d = nc.dram_tensor("xT", [DIM, N], F32R, kind="ExternalInput")
    wqkv_d = nc.dram_tensor("wqkv", [DIM, 3 * F], F32R, kind="ExternalInput")
    ahatT_d = nc.dram_tensor("ahatT", [N, N], F32R, kind="ExternalInput")
    wout_d = nc.dram_tensor("wout", [F, DIM], F32R, kind="ExternalInput")
    bout_d = nc.dram_tensor("bout", [1, DIM], F32, kind="ExternalInput")
    out_d = nc.dram_tensor("out", [N, DIM], F32, kind="ExternalOutput")

    def copy_out(dst, src):
        if copies_dve:
            nc.vector.tensor_copy(out=dst, in_=src)
        else:
            nc.scalar.copy(out=dst, in_=src)

    # PSUM budget is 8 banks total; a [128, 512] fp32 tile is one bank.
    if pair_heads:
        exp_batch = 1       # score tiles stay 1 bank; 2 in flight per jb
        s_bufs = 3 if s_bufs is None else s_bufs
        o_bufs = 3 if o_bufs is None else o_bufs
    else:
        s_bufs = 2 if s_bufs is None else s_bufs
        o_bufs = 2 if o_bufs is None else o_bufs
    with tile.TileContext(nc) as tc:
        with (
            tc.tile_pool(name="big", bufs=1) as big,
            tc.tile_pool(name="ps_mm", bufs=mm_bufs, space="PSUM") as ps_mm,
            tc.tile_pool(name="ps_s", bufs=s_bufs, space="PSUM") as ps_s,
            tc.tile_pool(name="ps_o", bufs=o_bufs, space="PSUM") as ps_o,
        ):
          for _rep in range(reps):
            # ---- persistent SBUF tensors -------------------------------
            ahatT = big.tile([128, NT, N], F32R)         # A_hat^T[j, i]
            wout = big.tile([128, FT, DIM], F32R)
            qkT = big.tile([128, 2 * FT, N], F32R)       # [f, n] f=q(0:512),k(512:1024)
            v_sb = big.tile([128, NT, F], F32R)          # v[n, f]
            vaug = big.tile([128, NT, H, DH + 1], F32R)  # [n, h, v|1]
            yT = big.tile([128, FT, N], F32R)            # Y^T[f, i]
            bout_bc = big.tile([128, DIM], F32)

            nc.scalar.dma_start(
                out=ahatT,
                in_=ahatT_d[:, :].rearrange("(t p) n -> p t n", p=128),
            )
            nc.scalar.dma_start(
                out=wout,
                in_=wout_d[:, :].rearrange("(t p) n -> p t n", p=128),
            )
            nc.sync.dma_start(out=bout_bc, in_=bout_d[0:1, :].to_broadcast((128, DIM)))
            nc.vector.memset(vaug.bitcast(F32), 1.0)  # ones col survives v copies

            # ---- phase 1: qT/kT (transposed) and v (natural) -----------
            with tc.tile_pool(name="ph1", bufs=1) as ph1:
                xT = ph1.tile([128, DT, N], F32R)        # xT[dim, n]
                wqkv = ph1.tile([128, DT, 3 * F], F32R)
                nc.sync.dma_start(
                    out=xT,
                    in_=xT_d[:, :].rearrange("(t p) n -> p t n", p=128),
                )
                nc.sync.dma_start(
                    out=wqkv,
                    in_=wqkv_d[:, :].rearrange("(t p) f -> p t f", p=128),
                )
                for ft in range(2 * FT):      # 8 tiles of q|k features
                    for c in range(NC2):
                        ps = ps_mm.tile([128, 512], F32, tag="mm")
                        for dt_i in range(DT):
                            nc.tensor.matmul(
                                ps,
                                wqkv[:, dt_i, ft * 128:(ft + 1) * 128],
                                xT[:, dt_i, c * 512:(c + 1) * 512],
                                start=(dt_i == 0),
                                stop=(dt_i == DT - 1),
                            )
                        copy_out(qkT[:, ft, c * 512:(c + 1) * 512], ps)

                for nt in range(NT):
                    ps = ps_mm.tile([128, 512], F32, tag="mm")
                    for dt_i in range(DT):
                        nc.tensor.matmul(
                            ps,
                            xT[:, dt_i, nt * 128:(nt + 1) * 128],
                            wqkv[:, dt_i, 2 * F:3 * F],
                            start=(dt_i == 0),
                            stop=(dt_i == DT - 1),
                        )
                    copy_out(v_sb[:, nt, :], ps)
                    nc.vector.tensor_copy(
                        out=vaug[:, nt, :, 0:DH],
                        in_=ps.rearrange("p (h d) -> p h d", h=H),
                    )

            exps = tc.alloc_tile_pool(name="exps", bufs=exps_bufs)
            small = tc.alloc_tile_pool(name="small", bufs=2)
            outs = tc.alloc_tile_pool(name="outs", bufs=2)
            dscr = tc.alloc_tile_pool(name="dscr", bufs=2, space="DRAM")

            def ahat_unit(ft, c):
                # (A_hat @ V)^T [f-tile ft, i-chunk c] -> yT
                ps = ps_mm.tile([128, 512], F32, tag="mm")
                for jt in range(NT):
                    nc.tensor.matmul(
                        ps,
                        v_sb[:, jt, ft * 128:(ft + 1) * 128],
                        ahatT[:, jt, c * 512:(c + 1) * 512],
                        start=(jt == 0),
                        stop=(jt == NT - 1),
                    )
                copy_out(yT[:, ft, c * 512:(c + 1) * 512], ps)

            def attn_tail(h, c, ps_out):
                # normalize expv^T by the ridden denominator and add into yT
                hb = (h % 2) * 64
                recip = small.tile([65, 512], F32, tag="recip")
                nc.vector.reciprocal(out=recip[64:65, :], in_=ps_out[64:65, :])
                # partition-broadcast via DRAM bounce (SBUF sources can't
                # broadcast across partitions; DRAM sources can)
                scr = dscr.tile([1, 512], F32, tag="scr")
                nc.sync.dma_start(out=scr, in_=recip[64:65, :])
                bcast = small.tile([64, 512], F32, tag="bcast")
                nc.sync.dma_start(out=bcast, in_=scr.to_broadcast((64, 512)))
                prod = small.tile([64, 512], F32R, tag="prod")
                nc.vector.tensor_mul(prod, ps_out[0:64, :], bcast)
                ysl = yT[hb:hb + 64, h // 2, c * 512:(c + 1) * 512]
                if hb == 0:
                    nc.vector.tensor_add(ysl, ysl, prod)
                else:
                    # DVE lanes can't shift partitions; SWDGE DMA-accumulate
                    # adds the base-0 product into the base-64 yT slice.
                    nc.gpsimd.dma_start(
                        out=ysl, in_=prod, accum_op=mybir.AluOpType.add,
                    )

            def attn_unit(h, c):
                # one head, one 512-wide i-chunk
                hb = (h % 2) * 64
                ht = h // 2
                ps_out = ps_o.tile([65, 512], F32, tag="po")
                for jb in range(NT // exp_batch):
                    ps_sc = ps_s.tile([128, exp_batch, 512], F32, tag="ps")
                    for e in range(exp_batch):
                        jt = jb * exp_batch + e
                        # scoresT[j, i] = sum_d kT[d, j] qT[d, i]
                        nc.tensor.matmul(
                            ps_sc[:, e, :],
                            qkT[hb:hb + 64, FT + ht, jt * 128:(jt + 1) * 128],
                            qkT[hb:hb + 64, ht, c * 512:(c + 1) * 512],
                        )
                    et = exps.tile([128, exp_batch, 512], F32R, tag="exp")
                    if skip_exp:
                        nc.vector.tensor_copy(out=et, in_=ps_sc)
                    else:
                        nc.scalar.activation(
                            out=et, in_=ps_sc,
                            func=mybir.ActivationFunctionType.Exp,
                            scale=float(SCALE),
                        )
                    for e in range(exp_batch):
                        jt = jb * exp_batch + e
                        # [expv^T ; denom] accumulation
                        nc.tensor.matmul(
                            ps_out,
                            vaug[:, jt, h, :],
                            et[:, e, :],
                            start=(jt == 0),
                            stop=(jt == NT - 1),
                        )
                attn_tail(h, c, ps_out)

            def attn_unit_pair(hp, c):
                # heads 2hp (rows 0:64) and 2hp+1 (rows 64:128) share qkT
                # tiles; their score matmuls hit disjoint PE row groups and
                # run concurrently.
                ht = hp
                po = [ps_o.tile([65, 512], F32, tag="po", name=f"po{u}")
                      for u in range(2)]
                for jb in range(NT // exp_batch):
                    ps_sc = [ps_s.tile([128, exp_batch, 512], F32, tag="ps",
                                       name=f"ps_sc{u}")
                             for u in range(2)]
                    for e in range(exp_batch):
                        jt = jb * exp_batch + e
                        for u, hb in enumerate((0, 64)):
                            nc.tensor.matmul(
                                ps_sc[u][:, e, :],
                                qkT[hb:hb + 64, FT + ht, jt * 128:(jt + 1) * 128],
                                qkT[hb:hb + 64, ht, c * 512:(c + 1) * 512],
                            )
                    ets = []
                    for u in range(2):
                        et = exps.tile([128, exp_batch, 512], F32R, tag="exp",
                                       name=f"et{u}")
                        if skip_exp:
                            nc.vector.tensor_copy(out=et, in_=ps_sc[u])
                        else:
                            nc.scalar.activation(
                                out=et, in_=ps_sc[u],
                                func=mybir.ActivationFunctionType.Exp,
                                scale=float(SCALE),
                            )
                        ets.append(et)
                    for e in range(exp_batch):
                        jt = jb * exp_batch + e
                        for u in range(2):
                            nc.tensor.matmul(
                                po[u],
                                vaug[:, jt, 2 * hp + u, :],
                                ets[u][:, e, :],
                                start=(jt == 0),
                                stop=(jt == NT - 1),
                            )
                for u in range(2):
                    attn_tail(2 * hp + u, c, po[u])

            # ---- phases 2+3: A_hat@V interleaved with attention --------
            units = []
            if pair_heads:
                for hp in range(H // 2):
                    for c in range(NC2):
                        if interleave:
                            units.append(("ahat", hp, c))
                        if not skip_attn:
                            units.append(("pair", hp, c))
                if not interleave:
                    units = [("ahat", ft, c) for ft in range(FT)
                             for c in range(NC2)] + units
            else:
                for h in range(H):
                    for c in range(NC2):
                        if interleave and h % 2 == 0:
                            units.append(("ahat", h // 2, c))
                        if not skip_attn:
                            units.append(("attn", h, c))
                if not interleave:
                    units = [("ahat", ft, c) for ft in range(FT)
                             for c in range(NC2)] + units

            for kind, a, c in units:
                if kind == "ahat":
                    ahat_unit(a, c)
                elif kind == "pair":
                    attn_unit_pair(a, c)
                else:
                    attn_unit(a, c)

            # ---- phase 4: out = Y @ w_out + b_out ----------------------
            for nt in range(NT):
                ps = ps_mm.tile([128, 512], F32, tag="mm")
                for ft in range(FT):
                    nc.tensor.matmul(
                        ps,
                        yT[:, ft, nt * 128:(nt + 1) * 128],
                        wout[:, ft, :],
                        start=(ft == 0),
                        stop=(ft == FT - 1),
                    )
                ot = outs.tile([128, DIM], F32, tag="ot")
                nc.vector.tensor_add(ot, ps, bout_bc)
                nc.sync.dma_start(out=out_d[nt * 128:(nt + 1) * 128, :], in_=ot)

            dscr.release()
            outs.release()
            small.release()
            exps.release()

    nc.compile()
    return nc


def _get_program():
    global _PROGRAM
    if _PROGRAM is None:
        _PROGRAM = _build_program()
    return _PROGRAM


def kernel(x, adj, w_qkv, w_out, b_out):
    x = np.asarray(x, dtype=np.float32)
    adj = np.asarray(adj, dtype=np.float32)
    w_qkv = np.ascontiguousarray(np.asarray(w_qkv, dtype=np.float32))
    w_out = np.ascontiguousarray(np.asarray(w_out, dtype=np.float32))
    b_out = np.asarray(b_out, dtype=np.float32).reshape(1, DIM)

    # host-side: normalized adjacency bias, replicated (cheap: one 1024^2 pass)
    A = np.ceil(adj) + np.eye(N, dtype=np.float32)
    dinv = A.sum(axis=1) ** -0.5
    A_hat = (A * dinv[:, None]) * dinv[None, :]
    ahatT = np.ascontiguousarray(A_hat.T)

    nc = _get_program()
    in_maps = []
    for b in range(B):
        in_maps.append({
            "xT": np.ascontiguousarray(x[b].T),
            "wqkv": w_qkv,
            "ahatT": ahatT,
            "wout": w_out,
            "bout": b_out,
        })
    global _last_in_maps
    _last_in_maps = in_maps
    res = run_bass_kernel_spmd(nc, in_maps, list(range(B)))
    out = np.stack([res.results[b]["out"] for b in range(B)], axis=0)
    return out.astype(np.float32)


if __name__ == "__main__":
    rng = np.random.default_rng(0)
    x = rng.standard_normal((B, N, DIM), dtype=np.float32)
    adj = (rng.random((N, N), dtype=np.float32) < 0.05).astype(np.float32) * 0.5
    w_qkv = rng.standard_normal((DIM, 3 * F), dtype=np.float32) * DIM ** -0.5
    w_out = rng.standard_normal((F, DIM), dtype=np.float32) * F ** -0.5
    b_out = np.zeros(DIM, dtype=np.float32)
    out = kernel(x=x, adj=adj, w_qkv=w_qkv, w_out=w_out, b_out=b_out)
    print("out", out.shape, out.dtype, np.abs(out).max())
